# revision 4
# baseline (speedup 1.0000x reference)
"""M2BertAttention (Monarch Mixer gated attention block) on 8 Trainium2 cores.

Math (per token row x of length H=768):
    mixed = monarch(x)  = x @ M          (M densified from the two Monarch
                                          block-diagonal factors on the host:
                                          M[(k,i),(c,q)] = w1[k,i,q]*w2[q,k,c])
    gate  = sigmoid(x @ gate_w.T + gate_b)
    h     = mixed * gate
    z     = h @ out_w.T + out_b + x
    out   = layernorm(z) * gamma + beta

Sharding: pure data parallel over the 16384 tokens -> 2048 tokens/core on 8
cores; all weights replicated. Per core the kernel runs 4 blocks of 512
tokens. The gate/monarch matmuls produce feature-major tiles (features on
partitions, tokens on the free dim) whose outputs (h^T) directly serve as the
stationary operand of the output projection, which lands token-major so the
residual + layernorm run with the feature reduction on the free dim. The
feature-major X^T operand is produced on the host (shipped alongside X) so no
on-chip transposes are needed.
"""

import numpy as np

import concourse.bass as bass
import concourse.mybir as mybir
import concourse.tile as tile
from concourse import bacc
from concourse import bass_utils

# Problem shape (hardcoded per the grading contract).
B, S, H = 4, 4096, 768
NB, BSZ = 16, 48
LN_EPS = 1e-12

N_CORES = 8
P = 128                  # partitions
KC = H // P              # 6 contraction chunks of 128
NTOK = B * S             # 16384 tokens total
NT_CORE = NTOK // N_CORES  # 2048 tokens per core
TBLK = 512               # tokens per block (matmul moving dim)
NBLK = NT_CORE // TBLK   # 4 blocks per core
TC = TBLK // P           # 4 token chunks of 128 per block
NSPLIT = (512, 256)      # output-projection free-dim split (PSUM bank limit)

F32 = mybir.dt.float32

_CACHE: dict = {}


def _build(mm_dt, use_ob: bool, use_gamma_beta: bool):
    """Build + compile the per-core Bass program."""
    nc = bacc.Bacc(
        "TRN2",
        target_bir_lowering=False,
        debug=False,
        enable_asserts=False,
        num_devices=N_CORES,
    )

    MDT = mm_dt
    xt_d = nc.dram_tensor("xt", [H, NT_CORE], MDT, kind="ExternalInput").ap()
    x_d = nc.dram_tensor("x", [NT_CORE, H], F32, kind="ExternalInput").ap()
    wg_d = nc.dram_tensor("wg", [H, H], MDT, kind="ExternalInput").ap()
    wm_d = nc.dram_tensor("wm", [H, H], MDT, kind="ExternalInput").ap()
    wo_d = nc.dram_tensor("wo", [H, H], MDT, kind="ExternalInput").ap()
    gb_d = nc.dram_tensor("gb", [P, KC], F32, kind="ExternalInput").ap()
    if use_ob:
        ob_d = nc.dram_tensor("ob", [1, H], MDT, kind="ExternalInput").ap()
    if use_gamma_beta:
        gam_d = nc.dram_tensor("gam", [1, H], F32, kind="ExternalInput").ap()
        bet_d = nc.dram_tensor("bet", [1, H], F32, kind="ExternalInput").ap()
    y_d = nc.dram_tensor("y", [NT_CORE, H], F32, kind="ExternalOutput").ap()

    with tile.TileContext(nc) as tc:
        with (
            tc.tile_pool(name="consts", bufs=1) as consts,
            tc.tile_pool(name="xtp", bufs=2) as xtp,
            tc.tile_pool(name="xp", bufs=2) as xp,
            tc.tile_pool(name="htp", bufs=2) as htp,
            tc.tile_pool(name="gtp", bufs=3) as gtp,
            tc.tile_pool(name="zp", bufs=3) as zp,
            tc.tile_pool(name="ystp", bufs=2) as ystp,
            tc.tile_pool(name="statp", bufs=4) as statp,
            tc.tile_pool(name="gpsp", bufs=2, space="PSUM") as gpsp,
            tc.tile_pool(name="mpsp", bufs=2, space="PSUM") as mpsp,
            tc.tile_pool(name="ops1p", bufs=2, space="PSUM") as ops1p,
            tc.tile_pool(name="ops2p", bufs=2, space="PSUM") as ops2p,
        ):
            # ---- constants -------------------------------------------------
            wg_sb = consts.tile([P, KC, H], MDT)
            wm_sb = consts.tile([P, KC, H], MDT)
            wo_sb = consts.tile([P, KC, H], MDT)
            # per-k DMAs so the first matmuls can start before all chunks land
            for k in range(KC):
                nc.sync.dma_start(
                    out=wg_sb[:, k, :], in_=wg_d[k * P:(k + 1) * P, :]
                )
                nc.sync.dma_start(
                    out=wm_sb[:, k, :], in_=wm_d[k * P:(k + 1) * P, :]
                )
            for k in range(KC):
                nc.sync.dma_start(
                    out=wo_sb[:, k, :], in_=wo_d[k * P:(k + 1) * P, :]
                )
            gb_sb = consts.tile([P, KC], F32)
            nc.sync.dma_start(out=gb_sb[:], in_=gb_d[:])
            eps_sb = consts.tile([P, 1], F32)
            nc.vector.memset(eps_sb, LN_EPS)
            if use_ob:
                ob_sb = consts.tile([1, H], MDT)
                nc.sync.dma_start(out=ob_sb[:], in_=ob_d[:])
                ones_sb = consts.tile([1, P], MDT)
                nc.vector.memset(ones_sb, 1.0)
            if use_gamma_beta:
                gam_sb = consts.tile([P, H], F32)
                bet_sb = consts.tile([P, H], F32)
                nc.sync.dma_start(
                    out=gam_sb[:],
                    in_=bass.AP(
                        tensor=gam_d.tensor, offset=gam_d.offset,
                        ap=[[0, P], [1, H]],
                    ),
                )
                nc.sync.dma_start(
                    out=bet_sb[:],
                    in_=bass.AP(
                        tensor=bet_d.tensor, offset=bet_d.offset,
                        ap=[[0, P], [1, H]],
                    ),
                )

            ht_tiles = [None] * NBLK
            x_tiles = [None] * NBLK

            def phase_a(b):
                """Gate + monarch matmuls for block b, feature-major."""
                xt_sb = xtp.tile([P, KC, TBLK], MDT, name=f"xt_{b}", tag="xt")
                nc.sync.dma_start(
                    out=xt_sb[:],
                    in_=xt_d[:, b * TBLK:(b + 1) * TBLK].rearrange(
                        "(k p) t -> p k t", p=P
                    ),
                )
                x_sb = xp.tile([P, TC, H], F32, name=f"x_{b}", tag="x")
                nc.sync.dma_start(
                    out=x_sb[:],
                    in_=x_d[b * TBLK:(b + 1) * TBLK, :].rearrange(
                        "(c p) h -> p c h", p=P
                    ),
                )
                x_tiles[b] = x_sb
                ht_sb = htp.tile([P, KC, TBLK], MDT, name=f"ht_{b}", tag="ht")
                ht_tiles[b] = ht_sb
                for j in range(KC):
                    g_ps = gpsp.tile([P, TBLK], F32, name=f"g_ps_{b}_{j}", tag="gps")
                    for k in range(KC):
                        nc.tensor.matmul(
                            g_ps[:],
                            wg_sb[:, k, j * P:(j + 1) * P],
                            xt_sb[:, k, :],
                            start=(k == 0),
                            stop=(k == KC - 1),
                        )
                    gt_sb = gtp.tile([P, TBLK], F32, name=f"gt_{b}_{j}", tag="gt")
                    nc.scalar.activation(
                        out=gt_sb[:],
                        in_=g_ps[:],
                        func=mybir.ActivationFunctionType.Sigmoid,
                        bias=gb_sb[:, j:j + 1],
                        scale=1.0,
                    )
                    m_ps = mpsp.tile([P, TBLK], F32, name=f"m_ps_{b}_{j}", tag="mps")
                    for k in range(KC):
                        nc.tensor.matmul(
                            m_ps[:],
                            wm_sb[:, k, j * P:(j + 1) * P],
                            xt_sb[:, k, :],
                            start=(k == 0),
                            stop=(k == KC - 1),
                        )
                    nc.vector.tensor_mul(ht_sb[:, j, :], m_ps[:], gt_sb[:])

            def phase_b(b):
                """Output projection + residual + layernorm for block b."""
                ht_sb = ht_tiles[b]
                x_sb = x_tiles[b]
                yst = ystp.tile([P, TC, H], F32, name=f"yst_{b}", tag="yst")
                for c in range(TC):
                    o_ps1 = ops1p.tile(
                        [P, NSPLIT[0]], F32, name=f"o1_{b}_{c}", tag="o1"
                    )
                    o_ps2 = ops2p.tile(
                        [P, NSPLIT[1]], F32, name=f"o2_{b}_{c}", tag="o2"
                    )
                    for k in range(KC):
                        nc.tensor.matmul(
                            o_ps1[:],
                            ht_sb[:, k, c * P:(c + 1) * P],
                            wo_sb[:, k, 0:NSPLIT[0]],
                            start=(k == 0),
                            stop=(k == KC - 1 and not use_ob),
                        )
                    if use_ob:
                        nc.tensor.matmul(
                            o_ps1[:],
                            ones_sb[:],
                            ob_sb[:, 0:NSPLIT[0]],
                            start=False,
                            stop=True,
                        )
                    for k in range(KC):
                        nc.tensor.matmul(
                            o_ps2[:],
                            ht_sb[:, k, c * P:(c + 1) * P],
                            wo_sb[:, k, NSPLIT[0]:H],
                            start=(k == 0),
                            stop=(k == KC - 1 and not use_ob),
                        )
                    if use_ob:
                        nc.tensor.matmul(
                            o_ps2[:],
                            ones_sb[:],
                            ob_sb[:, NSPLIT[0]:H],
                            start=False,
                            stop=True,
                        )
                    # residual add (z = proj + x), token-major
                    z_sb = zp.tile([P, H], F32, name=f"z_{b}_{c}", tag="z")
                    nc.vector.tensor_add(
                        z_sb[:, 0:NSPLIT[0]], o_ps1[:], x_sb[:, c, 0:NSPLIT[0]]
                    )
                    nc.vector.tensor_add(
                        z_sb[:, NSPLIT[0]:H], o_ps2[:], x_sb[:, c, NSPLIT[0]:H]
                    )
                    # layernorm stats over the 768 free elems (3 x 256)
                    stats = statp.tile([P, 3, 6], F32, name=f"st_{b}_{c}", tag="st")
                    z_r = z_sb.rearrange("p (s d) -> p s d", d=256)
                    for s in range(3):
                        nc.vector.bn_stats(out=stats[:, s, :], in_=z_r[:, s, :])
                    mv = statp.tile([P, 2], F32, name=f"mv_{b}_{c}", tag="mv")
                    nc.vector.bn_aggr(out=mv[:], in_=stats[:])
                    rs = statp.tile([P, 1], F32, name=f"rs_{b}_{c}", tag="rs")
                    nc.scalar.activation(
                        out=rs[:],
                        in_=mv[:, 1:2],
                        func=mybir.ActivationFunctionType.Sqrt,
                        bias=eps_sb[:, 0:1],
                        scale=1.0,
                    )
                    nc.vector.reciprocal(out=rs[:], in_=rs[:])
                    nm = statp.tile([P, 1], F32, name=f"nm_{b}_{c}", tag="nm")
                    nc.vector.scalar_tensor_tensor(
                        out=nm[:],
                        in0=mv[:, 0:1],
                        scalar=-1.0,
                        in1=rs[:],
                        op0=mybir.AluOpType.mult,
                        op1=mybir.AluOpType.mult,
                    )
                    if use_gamma_beta:
                        t_sb = zp.tile([P, H], F32, name=f"t_{b}_{c}", tag="z")
                        nc.scalar.activation(
                            out=t_sb[:],
                            in_=z_sb[:],
                            func=mybir.ActivationFunctionType.Identity,
                            bias=nm[:, 0:1],
                            scale=rs[:, 0:1],
                        )
                        nc.vector.tensor_mul(t_sb[:], t_sb[:], gam_sb[:])
                        nc.vector.tensor_add(yst[:, c, :], t_sb[:], bet_sb[:])
                    else:
                        nc.scalar.activation(
                            out=yst[:, c, :],
                            in_=z_sb[:],
                            func=mybir.ActivationFunctionType.Identity,
                            bias=nm[:, 0:1],
                            scale=rs[:, 0:1],
                        )
                nc.sync.dma_start(
                    out=y_d[b * TBLK:(b + 1) * TBLK, :].rearrange(
                        "(c p) h -> p c h", p=P
                    ),
                    in_=yst[:],
                )

            # software-pipelined: emit block b's gate/monarch matmuls before
            # block b-1's output projection so the PE never waits on the
            # sigmoid/mul of the block it just produced
            for step in range(NBLK + 1):
                if step < NBLK:
                    phase_a(step)
                if step >= 1:
                    phase_b(step - 1)

    nc.compile()
    return nc


def _get_nc(mm_dt, use_ob, use_gamma_beta):
    key = (str(mm_dt), use_ob, use_gamma_beta)
    if key not in _CACHE:
        _CACHE[key] = _build(mm_dt, use_ob, use_gamma_beta)
    return _CACHE[key]


# Matmul input dtype: float32r streams at 4x the rate of float32 on the PE
# with fp32 storage (reduced-precision multiply, fp32 accumulate).
MM_DT = mybir.dt.float32r


def _host_prep(hidden_states, w1_blocks, w2_blocks, gate_w, gate_b,
               out_w, out_b, ln_gamma, ln_beta):
    x = np.ascontiguousarray(
        np.asarray(hidden_states, dtype=np.float32).reshape(NTOK, H)
    )
    xt = np.ascontiguousarray(x.T)
    w1 = np.asarray(w1_blocks, dtype=np.float32)
    w2 = np.asarray(w2_blocks, dtype=np.float32)
    # dense monarch matrix: M[(k,i),(c,q)] = w1[k,i,q] * w2[q,k,c]
    M = np.einsum("kiq,qkc->kicq", w1, w2).reshape(H, H)
    wg = np.ascontiguousarray(np.asarray(gate_w, dtype=np.float32).T)
    wo = np.ascontiguousarray(np.asarray(out_w, dtype=np.float32).T)
    gb = np.ascontiguousarray(
        np.asarray(gate_b, dtype=np.float32).reshape(KC, P).T
    )
    ob = np.asarray(out_b, dtype=np.float32).reshape(1, H)
    gam = np.asarray(ln_gamma, dtype=np.float32).reshape(1, H)
    bet = np.asarray(ln_beta, dtype=np.float32).reshape(1, H)

    use_ob = bool(np.any(ob))
    use_gamma_beta = bool(np.any(gam != 1.0) or np.any(bet))

    in_maps = []
    for c in range(N_CORES):
        m = {
            "xt": np.ascontiguousarray(xt[:, c * NT_CORE:(c + 1) * NT_CORE]),
            "x": x[c * NT_CORE:(c + 1) * NT_CORE, :],
            "wg": wg,
            "wm": M,
            "wo": wo,
            "gb": gb,
        }
        if use_ob:
            m["ob"] = ob
        if use_gamma_beta:
            m["gam"] = gam
            m["bet"] = bet
        in_maps.append(m)
    return in_maps, use_ob, use_gamma_beta


def kernel(hidden_states, w1_blocks, w2_blocks, gate_w, gate_b,
           out_w, out_b, ln_gamma, ln_beta):
    in_maps, use_ob, use_gamma_beta = _host_prep(
        hidden_states, w1_blocks, w2_blocks, gate_w, gate_b,
        out_w, out_b, ln_gamma, ln_beta,
    )
    nc = _get_nc(MM_DT, use_ob, use_gamma_beta)
    res = bass_utils.run_bass_kernel_spmd(
        nc, in_maps, core_ids=list(range(N_CORES))
    )
    y = np.concatenate([res.results[c]["y"] for c in range(N_CORES)], axis=0)
    return y.reshape(B, S, H)


# revision 13
# speedup vs baseline: 602.9253x; 602.9253x over previous
"""M2BertAttention (Monarch Mixer gated attention block) on 8 Trainium2 cores.

Math (per token row x of length H=768):
    mixed = monarch(x)  = x @ M          (M densified from the two Monarch
                                          block-diagonal factors on the host:
                                          M[(k,i),(c,q)] = w1[k,i,q]*w2[q,k,c])
    gate  = sigmoid(x @ gate_w.T + gate_b)
    h     = mixed * gate
    z     = h @ out_w.T + out_b + x
    out   = layernorm(z) * gamma + beta

Sharding: pure data parallel over the 16384 tokens -> 2048 tokens/core on 8
cores; all weights replicated. Per core the kernel runs 4 blocks of 512
tokens. The gate/monarch matmuls produce feature-major tiles (features on
partitions, tokens on the free dim) whose outputs (h^T) directly serve as the
stationary operand of the output projection, which lands token-major so the
residual + layernorm run with the feature reduction on the free dim. The
feature-major X^T operand is produced on the host (shipped alongside X) so no
on-chip transposes are needed.
"""

import numpy as np

import concourse.bass as bass
import concourse.mybir as mybir
import concourse.tile as tile
from concourse import bacc
from concourse import bass_utils

# Problem shape (hardcoded per the grading contract).
B, S, H = 4, 4096, 768
NB, BSZ = 16, 48
LN_EPS = 1e-12

N_CORES = 8
P = 128                  # partitions
KC = H // P              # 6 contraction chunks of 128
NTOK = B * S             # 16384 tokens total
NT_CORE = NTOK // N_CORES  # 2048 tokens per core
TBLK = 512               # tokens per block (matmul moving dim)
NBLK = NT_CORE // TBLK   # 4 blocks per core
TC = TBLK // P           # 4 token chunks of 128 per block
NSPLIT = (512, 256)      # output-projection free-dim split (PSUM bank limit)

F32 = mybir.dt.float32

_CACHE: dict = {}


def _build(mm_dt, use_ob: bool, use_gamma_beta: bool, reps: int = 1,
           loop_n: int | None = None, ablate: str = ""):
    """Build + compile the per-core Bass program."""
    nc = bacc.Bacc(
        "TRN2",
        target_bir_lowering=False,
        debug=False,
        enable_asserts=False,
        num_devices=N_CORES,
    )

    MDT = mm_dt
    xt_d = nc.dram_tensor("xt", [H, NT_CORE], MDT, kind="ExternalInput").ap()
    x_d = nc.dram_tensor("x", [NT_CORE, H], F32, kind="ExternalInput").ap()
    wg_d = nc.dram_tensor("wg", [H, H], MDT, kind="ExternalInput").ap()
    wm_d = nc.dram_tensor("wm", [H, H], MDT, kind="ExternalInput").ap()
    wo_d = nc.dram_tensor("wo", [H, H], MDT, kind="ExternalInput").ap()
    gb_d = nc.dram_tensor("gb", [P, KC], F32, kind="ExternalInput").ap()
    if use_ob:
        ob_d = nc.dram_tensor("ob", [1, H], MDT, kind="ExternalInput").ap()
    if use_gamma_beta:
        gam_d = nc.dram_tensor("gam", [1, H], F32, kind="ExternalInput").ap()
        bet_d = nc.dram_tensor("bet", [1, H], F32, kind="ExternalInput").ap()
    y_d = nc.dram_tensor("y", [NT_CORE, H], F32, kind="ExternalOutput").ap()

    with tile.TileContext(nc) as tc:
        with (
            tc.tile_pool(name="consts", bufs=1) as consts,
            tc.tile_pool(name="xtp", bufs=2) as xtp,
            tc.tile_pool(name="xp", bufs=2) as xp,
            tc.tile_pool(name="htp", bufs=2) as htp,
            tc.tile_pool(name="gtp", bufs=3) as gtp,
            tc.tile_pool(name="gtb", bufs=1) as gtb,
            tc.tile_pool(name="zp", bufs=3) as zp,
            tc.tile_pool(name="ystp", bufs=2) as ystp,
            tc.tile_pool(name="statp", bufs=4) as statp,
            tc.tile_pool(name="gpsp", bufs=2, space="PSUM") as gpsp,
            tc.tile_pool(name="mpsp", bufs=2, space="PSUM") as mpsp,
            tc.tile_pool(name="ops1p", bufs=2, space="PSUM") as ops1p,
            tc.tile_pool(name="ops2p", bufs=2, space="PSUM") as ops2p,
        ):
            # ---- constants -------------------------------------------------
            wg_sb = consts.tile([P, KC, H], MDT)
            wm_sb = consts.tile([P, KC, H], MDT)
            wo_sb = consts.tile([P, KC, H], MDT)
            gb_sb = consts.tile([P, KC], F32)
            nc.sync.dma_start(out=gb_sb[:], in_=gb_d[:])
            if loop_n is not None:
                for k in range(KC):
                    nc.sync.dma_start(
                        out=wg_sb[:, k, :], in_=wg_d[k * P:(k + 1) * P, :])
                    nc.sync.dma_start(
                        out=wm_sb[:, k, :], in_=wm_d[k * P:(k + 1) * P, :])
                    nc.sync.dma_start(
                        out=wo_sb[:, k, :], in_=wo_d[k * P:(k + 1) * P, :])
            eps_sb = consts.tile([P, 1], F32)
            nc.vector.memset(eps_sb, LN_EPS)
            if use_ob:
                ob_sb = consts.tile([1, H], MDT)
                nc.sync.dma_start(out=ob_sb[:], in_=ob_d[:])
                ones_sb = consts.tile([1, P], MDT)
                nc.vector.memset(ones_sb, 1.0)
            if use_gamma_beta:
                gam_sb = consts.tile([P, H], F32)
                bet_sb = consts.tile([P, H], F32)
                nc.sync.dma_start(
                    out=gam_sb[:],
                    in_=bass.AP(
                        tensor=gam_d.tensor, offset=gam_d.offset,
                        ap=[[0, P], [1, H]],
                    ),
                )
                nc.sync.dma_start(
                    out=bet_sb[:],
                    in_=bass.AP(
                        tensor=bet_d.tensor, offset=bet_d.offset,
                        ap=[[0, P], [1, H]],
                    ),
                )

            ht_tiles = [None] * NBLK
            x_tiles = [None] * NBLK

            def phase_a(b, load_weights, tag):
                """Gate + monarch matmuls for block b, feature-major."""
                if ablate == "noxdma":
                    x_tiles[b] = x_const
                    ht_sb = htp.tile([P, KC, TBLK], MDT, name=f"ht_{tag}", tag="ht")
                    ht_tiles[b] = ht_sb
                    phase_a_mms(xt_const, ht_sb, tag)
                    return
                xt_sb = xtp.tile([P, KC, TBLK], MDT, name=f"xt_{tag}", tag="xt")
                for k in range(KC):
                    # per-k loads so matmul (j=0, k) starts as soon as its
                    # chunk lands instead of waiting for the whole block
                    nc.sync.dma_start(
                        out=xt_sb[:, k, :],
                        in_=xt_d[k * P:(k + 1) * P, b * TBLK:(b + 1) * TBLK],
                    )
                    if load_weights:
                        # interleave the weight chunks with block 0's input
                        # so the PE starts after ~0.5 MB instead of ~8.6 MB
                        nc.sync.dma_start(
                            out=wg_sb[:, k, :], in_=wg_d[k * P:(k + 1) * P, :]
                        )
                if load_weights:
                    for k in range(KC):
                        nc.sync.dma_start(
                            out=wm_sb[:, k, :], in_=wm_d[k * P:(k + 1) * P, :]
                        )
                x_sb = xp.tile([P, TC, H], F32, name=f"x_{tag}", tag="x")
                nc.sync.dma_start(
                    out=x_sb[:],
                    in_=x_d[b * TBLK:(b + 1) * TBLK, :].rearrange(
                        "(c p) h -> p c h", p=P
                    ),
                )
                if load_weights:
                    for k in range(KC):
                        nc.sync.dma_start(
                            out=wo_sb[:, k, :], in_=wo_d[k * P:(k + 1) * P, :]
                        )
                x_tiles[b] = x_sb
                ht_sb = htp.tile([P, KC, TBLK], MDT, name=f"ht_{tag}", tag="ht")
                ht_tiles[b] = ht_sb
                phase_a_mms(xt_sb, ht_sb, tag, load_weights)

            def phase_a_mms(xt_sb, ht_sb, tag, load_weights=False):
                def gate_j(j, gt_out):
                    g_ps = gpsp.tile([P, TBLK], F32, name=f"g_ps_{tag}_{j}", tag="gps")
                    for k in range(KC):
                        nc.tensor.matmul(
                            g_ps[:],
                            wg_sb[:, k, j * P:(j + 1) * P],
                            xt_sb[:, k, :],
                            start=(k == 0),
                            stop=(k == KC - 1),
                        )
                    nc.scalar.activation(
                        out=gt_out,
                        in_=g_ps[:],
                        func=mybir.ActivationFunctionType.Sigmoid,
                        bias=gb_sb[:, j:j + 1],
                        scale=1.0,
                    )

                def monarch_j(j, gt_in):
                    m_ps = mpsp.tile([P, TBLK], F32, name=f"m_ps_{tag}_{j}", tag="mps")
                    for k in range(KC):
                        nc.tensor.matmul(
                            m_ps[:],
                            wm_sb[:, k, j * P:(j + 1) * P],
                            xt_sb[:, k, :],
                            start=(k == 0),
                            stop=(k == KC - 1),
                        )
                    nc.vector.tensor_mul(ht_sb[:, j, :], m_ps[:], gt_in)

                if load_weights:
                    # block 0 is DMA-paced: run all gate groups (only need
                    # wg) before any monarch group so the PE isn't stalled
                    # on the wm chunks still streaming in
                    gt_blk = gtb.tile([P, KC, TBLK], F32, name=f"gtb_{tag}", tag="gtb")
                    for j in range(KC):
                        gate_j(j, gt_blk[:, j, :])
                    for j in range(KC):
                        monarch_j(j, gt_blk[:, j, :])
                else:
                    for j in range(KC):
                        gt_sb = gtp.tile([P, TBLK], F32, name=f"gt_{tag}_{j}", tag="gt")
                        gate_j(j, gt_sb[:])
                        monarch_j(j, gt_sb[:])

            pb_ctr = [0]

            def phase_b(b):
                """Output projection + residual + layernorm for block b."""
                u = pb_ctr[0]
                pb_ctr[0] += 1
                ht_sb = ht_tiles[b]
                x_sb = x_tiles[b]
                yst = ystp.tile([P, TC, H], F32, name=f"yst_{u}", tag="yst")
                for c in range(TC):
                    o_ps1 = ops1p.tile(
                        [P, NSPLIT[0]], F32, name=f"o1_{u}_{c}", tag="o1"
                    )
                    o_ps2 = ops2p.tile(
                        [P, NSPLIT[1]], F32, name=f"o2_{u}_{c}", tag="o2"
                    )
                    for k in range(KC):
                        nc.tensor.matmul(
                            o_ps1[:],
                            ht_sb[:, k, c * P:(c + 1) * P],
                            wo_sb[:, k, 0:NSPLIT[0]],
                            start=(k == 0),
                            stop=(k == KC - 1 and not use_ob),
                        )
                    if use_ob:
                        nc.tensor.matmul(
                            o_ps1[:],
                            ones_sb[:],
                            ob_sb[:, 0:NSPLIT[0]],
                            start=False,
                            stop=True,
                        )
                    for k in range(KC):
                        nc.tensor.matmul(
                            o_ps2[:],
                            ht_sb[:, k, c * P:(c + 1) * P],
                            wo_sb[:, k, NSPLIT[0]:H],
                            start=(k == 0),
                            stop=(k == KC - 1 and not use_ob),
                        )
                    if use_ob:
                        nc.tensor.matmul(
                            o_ps2[:],
                            ones_sb[:],
                            ob_sb[:, NSPLIT[0]:H],
                            start=False,
                            stop=True,
                        )
                    # residual add (z = proj + x), token-major
                    z_sb = zp.tile([P, H], F32, name=f"z_{u}_{c}", tag="z")
                    nc.vector.tensor_add(
                        z_sb[:, 0:NSPLIT[0]], o_ps1[:], x_sb[:, c, 0:NSPLIT[0]]
                    )
                    nc.vector.tensor_add(
                        z_sb[:, NSPLIT[0]:H], o_ps2[:], x_sb[:, c, NSPLIT[0]:H]
                    )
                    if ablate == "noln":
                        nc.scalar.activation(
                            out=yst[:, c, :],
                            in_=z_sb[:],
                            func=mybir.ActivationFunctionType.Copy,
                        )
                        nc.sync.dma_start(
                            out=y_d[b * TBLK + c * P:b * TBLK + (c + 1) * P, :],
                            in_=yst[:, c, :],
                        )
                        continue
                    # layernorm stats over the 768 free elems (3 x 256)
                    stats = statp.tile([P, 3, 6], F32, name=f"st_{u}_{c}", tag="st")
                    z_r = z_sb.rearrange("p (s d) -> p s d", d=256)
                    for s in range(3):
                        nc.vector.bn_stats(out=stats[:, s, :], in_=z_r[:, s, :])
                    mv = statp.tile([P, 2], F32, name=f"mv_{u}_{c}", tag="mv")
                    nc.vector.bn_aggr(out=mv[:], in_=stats[:])
                    rs = statp.tile([P, 1], F32, name=f"rs_{u}_{c}", tag="rs")
                    nc.scalar.activation(
                        out=rs[:],
                        in_=mv[:, 1:2],
                        func=mybir.ActivationFunctionType.Sqrt,
                        bias=eps_sb[:, 0:1],
                        scale=1.0,
                    )
                    nc.vector.reciprocal(out=rs[:], in_=rs[:])
                    nm = statp.tile([P, 1], F32, name=f"nm_{u}_{c}", tag="nm")
                    nc.vector.scalar_tensor_tensor(
                        out=nm[:],
                        in0=mv[:, 0:1],
                        scalar=-1.0,
                        in1=rs[:],
                        op0=mybir.AluOpType.mult,
                        op1=mybir.AluOpType.mult,
                    )
                    if use_gamma_beta:
                        t_sb = zp.tile([P, H], F32, name=f"t_{u}_{c}", tag="z")
                        nc.scalar.activation(
                            out=t_sb[:],
                            in_=z_sb[:],
                            func=mybir.ActivationFunctionType.Identity,
                            bias=nm[:, 0:1],
                            scale=rs[:, 0:1],
                        )
                        nc.vector.tensor_mul(t_sb[:], t_sb[:], gam_sb[:])
                        nc.vector.tensor_add(yst[:, c, :], t_sb[:], bet_sb[:])
                    else:
                        nc.scalar.activation(
                            out=yst[:, c, :],
                            in_=z_sb[:],
                            func=mybir.ActivationFunctionType.Identity,
                            bias=nm[:, 0:1],
                            scale=rs[:, 0:1],
                        )
                    # stream each 128-token chunk out as soon as its LN lands
                    nc.sync.dma_start(
                        out=y_d[b * TBLK + c * P:b * TBLK + (c + 1) * P, :],
                        in_=yst[:, c, :],
                    )

            # software-pipelined: emit block b's gate/monarch matmuls before
            # block b-1's output projection so the PE never waits on the
            # sigmoid/mul of the block it just produced. reps>1 repeats the
            # whole program body for steady-state HW timing measurements.
            xt_const = x_const = None
            if ablate == "noxdma":
                xt_const = consts.tile([P, KC, TBLK], MDT)
                for k in range(KC):
                    nc.sync.dma_start(
                        out=xt_const[:, k, :], in_=xt_d[k * P:(k + 1) * P, 0:TBLK])
                x_const = consts.tile([P, TC, H], F32)
                nc.sync.dma_start(
                    out=x_const[:],
                    in_=x_d[0:TBLK, :].rearrange("(c p) h -> p c h", p=P))

            dummy_y = None
            if ablate == "dma":
                dummy_y = consts.tile([P, TC, H], F32)
                nc.vector.memset(dummy_y[:, 0, 0:8], 0.0)

            def body_dma_only(r):
                for b in range(NBLK):
                    xt_sb = xtp.tile([P, KC, TBLK], MDT, name=f"xt_{r}_{b}", tag="xt")
                    for k in range(KC):
                        nc.sync.dma_start(
                            out=xt_sb[:, k, :],
                            in_=xt_d[k * P:(k + 1) * P, b * TBLK:(b + 1) * TBLK],
                        )
                    x_sb = xp.tile([P, TC, H], F32, name=f"x_{r}_{b}", tag="x")
                    nc.sync.dma_start(
                        out=x_sb[:],
                        in_=x_d[b * TBLK:(b + 1) * TBLK, :].rearrange(
                            "(c p) h -> p c h", p=P
                        ),
                    )
                    for c in range(TC):
                        nc.sync.dma_start(
                            out=y_d[b * TBLK + c * P:b * TBLK + (c + 1) * P, :],
                            in_=dummy_y[:, c, :],
                        )

            def body(r, load_w):
                if ablate == "dma":
                    body_dma_only(r)
                    return
                for step in range(NBLK + 1):
                    if step < NBLK:
                        phase_a(step, load_weights=(load_w and step == 0),
                                tag=f"{r}_{step}")
                    if step >= 1:
                        phase_b(step - 1)

            if loop_n is not None:
                # timing mode: loop the whole body on-device so the NEFF runs
                # long enough to dominate host-side measurement noise
                with tc.For_i(0, loop_n, 1,
                              hint_engines=(mybir.EngineType.PE,)):
                    body(0, False)
            else:
                for r in range(reps):
                    body(r, r == 0)

    nc.compile()
    return nc


def _get_nc(mm_dt, use_ob, use_gamma_beta, reps=1, loop_n=None, ablate=""):
    key = (str(mm_dt), use_ob, use_gamma_beta, reps, loop_n, ablate)
    if key not in _CACHE:
        _CACHE[key] = _build(mm_dt, use_ob, use_gamma_beta, reps, loop_n, ablate)
    return _CACHE[key]


# Matmul input dtype: float32r streams at 4x the rate of float32 on the PE
# with fp32 storage (reduced-precision multiply, fp32 accumulate).
MM_DT = mybir.dt.float16


def _host_prep(hidden_states, w1_blocks, w2_blocks, gate_w, gate_b,
               out_w, out_b, ln_gamma, ln_beta):
    x = np.ascontiguousarray(
        np.asarray(hidden_states, dtype=np.float32).reshape(NTOK, H)
    )
    xt = np.ascontiguousarray(x.T)
    w1 = np.asarray(w1_blocks, dtype=np.float32)
    w2 = np.asarray(w2_blocks, dtype=np.float32)
    # dense monarch matrix: M[(k,i),(c,q)] = w1[k,i,q] * w2[q,k,c]
    M = np.einsum("kiq,qkc->kicq", w1, w2).reshape(H, H)
    wg = np.ascontiguousarray(np.asarray(gate_w, dtype=np.float32).T)
    wo = np.ascontiguousarray(np.asarray(out_w, dtype=np.float32).T)
    gb = np.ascontiguousarray(
        np.asarray(gate_b, dtype=np.float32).reshape(KC, P).T
    )
    ob = np.asarray(out_b, dtype=np.float32).reshape(1, H)
    gam = np.asarray(ln_gamma, dtype=np.float32).reshape(1, H)
    bet = np.asarray(ln_beta, dtype=np.float32).reshape(1, H)

    use_ob = bool(np.any(ob))
    use_gamma_beta = bool(np.any(gam != 1.0) or np.any(bet))

    # matmul-side operands are stored in the matmul dtype
    if MM_DT == mybir.dt.float16:
        mm_np = np.float16
    elif MM_DT == mybir.dt.bfloat16:
        import ml_dtypes
        mm_np = ml_dtypes.bfloat16
    else:
        mm_np = np.float32
    xt = xt.astype(mm_np)
    wg = wg.astype(mm_np)
    M = M.astype(mm_np)
    wo = wo.astype(mm_np)
    ob = ob.astype(mm_np)

    in_maps = []
    for c in range(N_CORES):
        m = {
            "xt": np.ascontiguousarray(xt[:, c * NT_CORE:(c + 1) * NT_CORE]),
            "x": x[c * NT_CORE:(c + 1) * NT_CORE, :],
            "wg": wg,
            "wm": M,
            "wo": wo,
            "gb": gb,
        }
        if use_ob:
            m["ob"] = ob
        if use_gamma_beta:
            m["gam"] = gam
            m["bet"] = bet
        in_maps.append(m)
    return in_maps, use_ob, use_gamma_beta


def kernel(hidden_states, w1_blocks, w2_blocks, gate_w, gate_b,
           out_w, out_b, ln_gamma, ln_beta):
    in_maps, use_ob, use_gamma_beta = _host_prep(
        hidden_states, w1_blocks, w2_blocks, gate_w, gate_b,
        out_w, out_b, ln_gamma, ln_beta,
    )
    nc = _get_nc(MM_DT, use_ob, use_gamma_beta)
    res = bass_utils.run_bass_kernel_spmd(
        nc, in_maps, core_ids=list(range(N_CORES))
    )
    y = np.concatenate([res.results[c]["y"] for c in range(N_CORES)], axis=0)
    return y.reshape(B, S, H)
